# revision 1
# baseline (speedup 1.0000x reference)
"""Grouped triplet loss on 8 trn2 NeuronCores.

Strategy (data-parallel over A rows, hint-compliant):
  - Each core takes a 1024-row block of A, full B (column-rotated so the
    diagonal of the distance matrix lands at core-independent positions).
  - L2 normalization of A-block and B on device.
  - One fused matmul per (row-tile, col-chunk) computes the *masked* squared
    distance directly in PSUM via extended feature vectors:
        F_A = [ a_i (32) | 1 | -BIG*onehot(label_i) (32) ]   (K = 65)
        F_B = [ -2*b_j   | 2+BIG |      onehot(label_j)  ]
    so PSUM = 2 - 2*a.b + BIG*(1 - same_group).
  - A tiny bf16 identity matmul accumulates +BIG on the diagonal (self-pair
    exclusion).
  - DVE min-reduces PSUM (4 banks per op); rows with min >= TH had no valid
    negative -> dist_neg = 0 (matches torch "skip groups of size < 2").
  - losses = relu(dist_pos - dist_neg + margin); host averages.

Host-side work is limited to sharding/layout: slicing, row-rotation, (t p)
tiling, and one-hot encoding of the integer labels. All float math happens
on device.
"""

import numpy as np

import concourse.bass as bass
import concourse.mybir as mybir
from concourse.tile import TileContext
from concourse.bass_utils import run_bass_kernel_spmd

N, D, G = 8192, 32, 32
NCORES = 8
RPC = N // NCORES      # rows per core = 1024
RT = RPC // 128        # row tiles per core = 8
CT = N // 128          # column tiles = 64
NCHUNK = N // 512      # matmul column chunks = 16
BIG = 64.0
TH = 32.0
MARGIN = 1.0

F32 = mybir.dt.float32
BF16 = mybir.dt.bfloat16
AF = mybir.ActivationFunctionType
ALU = mybir.AluOpType
AX = mybir.AxisListType

MM_DT = mybir.dt.float32r  # matmul feature dtype (float32 | float32r)

_MAX_DRAIN_WAITS = 1


def _split_drain_waits(nc):
    """This container's walrus rejects any instruction with >1 sem-wait.
    Hoist excess waits onto preceding same-engine single-wait Drains."""
    nsplit = 0
    for f in nc.m.functions:
        for bb in f.blocks:
            new_insts = []
            for inst in bb.instructions:
                si = inst.sync_info
                waits = list(si.on_wait) if si and si.on_wait else []
                if len(waits) > _MAX_DRAIN_WAITS:
                    extra, keep = waits[:-_MAX_DRAIN_WAITS], waits[-_MAX_DRAIN_WAITS:]
                    for w in extra:
                        d = mybir.InstDrain(
                            name=f"{inst.name}-swsplit{nsplit}",
                            engine=inst.engine,
                            ins=[],
                            outs=[],
                            sync_info=mybir.SyncInfo(on_wait=[w], on_update=[]),
                        )
                        nsplit += 1
                        nc.register_instruction(d, overwrite=True)
                        new_insts.append(d)
                    si.on_wait = keep
                new_insts.append(inst)
            bb.instructions[:] = new_insts


def _build_nc():
    import ml_dtypes

    nc = bass.Bass()

    a_in = nc.dram_tensor("a", [128, RT * D], F32, kind="ExternalInput")
    b_in = nc.dram_tensor("b", [128, CT * D], F32, kind="ExternalInput")
    # row 0: constant feature (1 for A, 2+BIG for B); rows 1..32: one-hot
    oha_in = nc.dram_tensor("oha", [G + 1, RPC], MM_DT, kind="ExternalInput")
    ohb_in = nc.dram_tensor("ohb", [G + 1, N], MM_DT, kind="ExternalInput")
    out = nc.dram_tensor("losses", [128, RT], F32, kind="ExternalOutput")

    ident_np = np.eye(128, dtype=np.float32)
    sel_np = np.zeros((128, 1024), dtype=np.float32)
    sel_np[np.arange(128), 512 + np.arange(128)] = 1.0
    bigi_np = (BIG * np.eye(128)).astype(ml_dtypes.bfloat16)
    ident_d = nc.inline_tensor(ident_np, name="identc")
    sel_d = nc.inline_tensor(sel_np.astype(ml_dtypes.bfloat16), name="selc")
    bigi_d = nc.inline_tensor(bigi_np, name="bigic")

    with TileContext(nc) as tc:
        with (
            tc.tile_pool(name="const", bufs=1) as cpool,
            tc.tile_pool(name="work", bufs=1) as wpool,
            tc.tile_pool(name="ps", bufs=2, space="PSUM") as pspool,
        ):
            # ---- constants -------------------------------------------------
            ident = cpool.tile([128, 128], F32, tag="ident")
            nc.sync.dma_start(out=ident[:], in_=ident_d[:, :])
            sel = cpool.tile([128, 1024], BF16, tag="sel")
            nc.sync.dma_start(out=sel[:], in_=sel_d[:, :])
            bigi = cpool.tile([128, 128], BF16, tag="bigi")
            nc.sync.dma_start(out=bigi[:], in_=bigi_d[:, :])

            # ---- raw loads -------------------------------------------------
            tA = wpool.tile([128, RT * D], F32, tag="tA")
            nc.sync.dma_start(out=tA[:], in_=a_in[:, :])
            tB = wpool.tile([128, CT * D], F32, tag="tB")
            # split into 2 DMAs to use more queues
            nc.sync.dma_start(out=tB[:, : CT * D // 2], in_=b_in[:, : CT * D // 2])
            nc.sync.dma_start(out=tB[:, CT * D // 2 :], in_=b_in[:, CT * D // 2 :])

            fA = cpool.tile([G + 33, RPC], MM_DT, tag="fA")
            fB = cpool.tile([G + 33, N], MM_DT, tag="fB")
            nc.sync.dma_start(out=fA[32:65, :], in_=oha_in[:, :])
            nc.sync.dma_start(out=fB[32:65, : N // 2], in_=ohb_in[:, : N // 2])
            nc.sync.dma_start(out=fB[32:65, N // 2 :], in_=ohb_in[:, N // 2 :])

            # ---- normalize A block ----------------------------------------
            tA3 = tA[:, :].rearrange("p (t d) -> p t d", d=D)
            sqA = wpool.tile([128, RT * D], F32, tag="sqA")
            nc.scalar.activation(sqA[:], tA[:], AF.Square)
            ssA = wpool.tile([128, RT], F32, tag="ssA")
            nc.vector.tensor_reduce(
                ssA[:], sqA[:, :].rearrange("p (t d) -> p t d", d=D), axis=AX.X, op=ALU.add
            )
            nA = wpool.tile([128, RT], F32, tag="nA")
            nc.scalar.activation(nA[:], ssA[:], AF.Sqrt)
            rA = wpool.tile([128, RT], F32, tag="rA")
            nc.vector.reciprocal(rA[:], nA[:])
            an = wpool.tile([128, RT * D], F32, tag="an")
            an3 = an[:, :].rearrange("p (t d) -> p t d", d=D)
            nc.vector.tensor_tensor(
                an3, tA3, rA[:, :].broadcast_to([128, RT, D]), op=ALU.mult
            )

            # ---- normalize B (scaled by -2 for features) -------------------
            tB3 = tB[:, :].rearrange("p (t d) -> p t d", d=D)
            sqB = wpool.tile([128, CT * D], F32, tag="sqB")
            nc.scalar.activation(sqB[:], tB[:], AF.Square)
            ssB = wpool.tile([128, CT], F32, tag="ssB")
            nc.vector.tensor_reduce(
                ssB[:], sqB[:, :].rearrange("p (t d) -> p t d", d=D), axis=AX.X, op=ALU.add
            )
            nB = wpool.tile([128, CT], F32, tag="nB")
            nc.scalar.activation(nB[:], ssB[:], AF.Sqrt)
            rB = wpool.tile([128, CT], F32, tag="rB")
            nc.vector.reciprocal(rB[:], nB[:])
            rBm2 = wpool.tile([128, CT], F32, tag="rBm2")
            nc.vector.tensor_scalar(rBm2[:], rB[:], -2.0, None, op0=ALU.mult)
            bn2 = wpool.tile([128, CT * D], F32, tag="bn2")
            bn23 = bn2[:, :].rearrange("p (t d) -> p t d", d=D)
            nc.vector.tensor_tensor(
                bn23, tB3, rBm2[:, :].broadcast_to([128, CT, D]), op=ALU.mult
            )

            # ---- transpose an -> fA[0:32, :] ------------------------------
            psA = pspool.tile([32, RPC], F32, tag="ps")
            for r in range(RT):
                nc.tensor.transpose(psA[:, r * 128 : (r + 1) * 128], an3[:, r, :], ident[:])
            nc.scalar.copy(fA[0:32, :], psA[:, :])

            # ---- transpose bn2 -> fB[0:32, :] ------------------------------
            for grp in range(CT // 16):
                psB = pspool.tile([32, 16 * 128], F32, tag="ps")
                for k in range(16):
                    t = grp * 16 + k
                    nc.tensor.transpose(
                        psB[:, k * 128 : (k + 1) * 128], bn23[:, t, :], ident[:]
                    )
                nc.scalar.copy(fB[0:32, grp * 2048 : (grp + 1) * 2048], psB[:, :])

            # ---- dist_pos for own rows (first RT tiles of rotated B) ------
            bno = wpool.tile([128, RT * D], F32, tag="bno")
            bno3 = bno[:, :].rearrange("p (t d) -> p t d", d=D)
            nc.vector.tensor_tensor(
                bno3, tB3[:, 0:RT, :], rB[:, 0:RT].broadcast_to([128, RT, D]), op=ALU.mult
            )
            dd = wpool.tile([128, RT * D], F32, tag="dd")
            nc.vector.tensor_tensor(dd[:], an[:], bno[:], op=ALU.subtract)
            sqd = wpool.tile([128, RT * D], F32, tag="sqd")
            nc.scalar.activation(sqd[:], dd[:], AF.Square)
            dp2 = wpool.tile([128, RT], F32, tag="dp2")
            nc.vector.tensor_reduce(
                dp2[:], sqd[:, :].rearrange("p (t d) -> p t d", d=D), axis=AX.X, op=ALU.add
            )
            dpos = wpool.tile([128, RT], F32, tag="dpos")
            nc.scalar.activation(dpos[:], dp2[:], AF.Sqrt)

            # ---- main loop: fused matmul + masked min ----------------------
            mpart = wpool.tile([128, RT * 4], F32, tag="mpart")
            for r in range(RT):
                lhsT = fA[:, r * 128 : (r + 1) * 128]
                for q in range(4):
                    P4 = pspool.tile([128, 2048], F32, tag="ps")
                    for j in range(4):
                        c = q * 4 + j
                        is_diag = q == 0 and j == r // 4
                        nc.tensor.matmul(
                            P4[:, j * 512 : (j + 1) * 512],
                            lhsT,
                            fB[:, c * 512 : (c + 1) * 512],
                            start=True,
                            stop=not is_diag,
                        )
                        if is_diag:
                            off = (r % 4) * 128
                            nc.tensor.matmul(
                                P4[:, j * 512 : (j + 1) * 512],
                                bigi[:],
                                sel[:, 512 - off : 1024 - off],
                                start=False,
                                stop=True,
                            )
                    nc.vector.tensor_reduce(
                        mpart[:, r * 4 + q : r * 4 + q + 1],
                        P4[:, :].rearrange("p (f c) -> p f c", c=512),
                        axis=AX.XY,
                        op=ALU.min,
                    )

            # ---- finalize --------------------------------------------------
            m = wpool.tile([128, RT], F32, tag="m")
            nc.vector.tensor_reduce(
                m[:], mpart[:, :].rearrange("p (r q) -> p r q", q=4), axis=AX.X, op=ALU.min
            )
            mc = wpool.tile([128, RT], F32, tag="mc")
            nc.vector.tensor_scalar(mc[:], m[:], 0.0, None, op0=ALU.max)
            sn = wpool.tile([128, RT], F32, tag="sn")
            nc.scalar.activation(sn[:], mc[:], AF.Sqrt)
            valid = wpool.tile([128, RT], F32, tag="valid")
            nc.vector.tensor_scalar(valid[:], m[:], TH, None, op0=ALU.is_lt)
            dn = wpool.tile([128, RT], F32, tag="dn")
            nc.vector.tensor_tensor(dn[:], sn[:], valid[:], op=ALU.mult)
            pre = wpool.tile([128, RT], F32, tag="pre")
            nc.vector.tensor_tensor(pre[:], dpos[:], dn[:], op=ALU.subtract)
            losses = wpool.tile([128, RT], F32, tag="losses")
            nc.scalar.activation(losses[:], pre[:], AF.Relu, bias=MARGIN)
            nc.sync.dma_start(out=out[:, :], in_=losses[:])

    _split_drain_waits(nc)
    return nc


def _build_nc_sorted(gpc, padg):
    """Group-sorted variant: each core gets `gpc` whole groups, each padded to
    `padg` rows/cols. Only within-group blocks are computed (the masked min
    never needs cross-group pairs). Columns = the core's own rows, so the
    self-pair diagonal sits at block-local positions; it is excluded by an
    in-place +BIG*I add on the 128-wide diagonal slab before the min-reduce.
    Padded columns carry constant-feature 2+BIG -> always excluded.

    Structured as a per-group pipeline: transpose -> feature copy -> matmul ->
    diag add -> min reduce, so PE/ACT/DVE overlap across groups. The B chain
    is emitted first (it gates the feature build); dist_pos is emitted last
    (only needed by the finalize stage)."""
    assert padg <= 512 and padg % 128 == 0
    rmax = gpc * padg          # rows (and cols) per core
    rt = rmax // 128           # 128-row tiles per core
    tpg = padg // 128          # row tiles per group

    nc = bass.Bass()
    a_in = nc.dram_tensor("a", [128, rt * D], F32, kind="ExternalInput")
    b_in = nc.dram_tensor("b", [128, rt * D], F32, kind="ExternalInput")
    cv_in = nc.dram_tensor("cv", [2, rmax], MM_DT, kind="ExternalInput")
    out = nc.dram_tensor("losses", [128, rt], F32, kind="ExternalOutput")

    ident_np = np.eye(128, dtype=np.float32)
    seld_np = (BIG * np.eye(128)).astype(np.float32)
    ident_d = nc.inline_tensor(ident_np, name="identc")
    seld_d = nc.inline_tensor(seld_np, name="seldc")

    half = rt * D // 2

    with TileContext(nc) as tc:
        with (
            tc.tile_pool(name="const", bufs=1) as cpool,
            tc.tile_pool(name="work", bufs=1) as wpool,
            tc.tile_pool(name="pst", bufs=2, space="PSUM") as pstp,
            tc.tile_pool(name="psm", bufs=4, space="PSUM") as psmp,
        ):
            # input DMAs first, spread across otherwise-idle engine queues
            tB = wpool.tile([128, rt * D], F32, tag="tB")
            nc.sync.dma_start(out=tB[:, :half], in_=b_in[:, :half])
            nc.sync.dma_start(out=tB[:, half:], in_=b_in[:, half:])
            tA = wpool.tile([128, rt * D], F32, tag="tA")
            nc.gpsimd.dma_start(out=tA[:, :half], in_=a_in[:, :half])
            nc.gpsimd.dma_start(out=tA[:, half:], in_=a_in[:, half:])

            ident = cpool.tile([128, 128], F32, tag="ident")
            nc.scalar.dma_start(out=ident[:], in_=ident_d[:, :])
            seld = cpool.tile([128, 128], F32, tag="seld")
            nc.scalar.dma_start(out=seld[:], in_=seld_d[:, :])

            fA = cpool.tile([33, rmax], MM_DT, tag="fA")
            fB = cpool.tile([33, rmax], MM_DT, tag="fB")
            nc.scalar.dma_start(out=fB[32:33, :], in_=cv_in[1:2, :])
            nc.scalar.dma_start(out=fA[32:33, :], in_=cv_in[0:1, :])

            # fire the ACT table load immediately (contents irrelevant)
            warmup_act = wpool.tile([128, 8], F32, tag="warmup_act")
            nc.scalar.activation(warmup_act[:], warmup_act[:], AF.Square)

            # ---- B chain (critical: gates the feature build) ----
            tB3 = tB[:, :].rearrange("p (t d) -> p t d", d=D)
            sqB = wpool.tile([128, rt * D], F32, tag="sqB")
            nc.scalar.activation(sqB[:, :half], tB[:, :half], AF.Square)
            nc.scalar.activation(sqB[:, half:], tB[:, half:], AF.Square)
            ssB = wpool.tile([128, rt], F32, tag="ssB")
            nc.vector.tensor_reduce(
                ssB[:], sqB[:, :].rearrange("p (t d) -> p t d", d=D), axis=AX.X, op=ALU.add
            )
            nB = wpool.tile([128, rt], F32, tag="nB")
            nc.scalar.activation(nB[:], ssB[:], AF.Sqrt)
            rB = wpool.tile([128, rt], F32, tag="rB")
            nc.vector.reciprocal(rB[:], nB[:])
            rBm2 = wpool.tile([128, rt], F32, tag="rBm2")
            nc.vector.tensor_scalar(rBm2[:], rB[:], -2.0, None, op0=ALU.mult)
            bn2 = wpool.tile([128, rt * D], F32, tag="bn2")
            bn23 = bn2[:, :].rearrange("p (t d) -> p t d", d=D)
            nc.vector.tensor_tensor(
                bn23, tB3, rBm2[:, :].broadcast_to([128, rt, D]), op=ALU.mult
            )

            # ---- A chain ----
            tA3 = tA[:, :].rearrange("p (t d) -> p t d", d=D)
            sqA = wpool.tile([128, rt * D], F32, tag="sqA")
            nc.scalar.activation(sqA[:, :half], tA[:, :half], AF.Square)
            nc.scalar.activation(sqA[:, half:], tA[:, half:], AF.Square)
            ssA = wpool.tile([128, rt], F32, tag="ssA")
            nc.vector.tensor_reduce(
                ssA[:], sqA[:, :].rearrange("p (t d) -> p t d", d=D), axis=AX.X, op=ALU.add
            )
            nA = wpool.tile([128, rt], F32, tag="nA")
            nc.scalar.activation(nA[:], ssA[:], AF.Sqrt)
            rA = wpool.tile([128, rt], F32, tag="rA")
            nc.vector.reciprocal(rA[:], nA[:])
            an = wpool.tile([128, rt * D], F32, tag="an")
            an3 = an[:, :].rearrange("p (t d) -> p t d", d=D)
            nc.vector.tensor_tensor(
                an3, tA3, rA[:, :].broadcast_to([128, rt, D]), op=ALU.mult
            )

            # ---- PE warm-up: dummy transposes keyed to sqB so the HAM
            # clock-gate opens before the real transposes/matmuls arrive ----
            for w in range(16):
                pw = psmp.tile([128, 512], F32, tag="psm")
                nc.tensor.transpose(pw[:, 0:128], sqB[:, 0:128], ident[:])

            # ---- per-group pipeline ----
            mpart = wpool.tile([128, rt], F32, tag="mpart")
            for gl in range(gpc):
                base = gl * tpg
                cs = gl * padg
                psB = pstp.tile([32, padg], F32, tag="pstB")
                for r in range(tpg):
                    nc.tensor.transpose(
                        psB[:, r * 128 : (r + 1) * 128], bn23[:, base + r, :], ident[:]
                    )
                nc.scalar.copy(fB[0:32, cs : cs + padg], psB[:, :])
                psA = pstp.tile([32, padg], F32, tag="pstA")
                for r in range(tpg):
                    nc.tensor.transpose(
                        psA[:, r * 128 : (r + 1) * 128], an3[:, base + r, :], ident[:]
                    )
                nc.scalar.copy(fA[0:32, cs : cs + padg], psA[:, :])
                for r in range(tpg):
                    idx = base + r
                    off = r * 128
                    P = psmp.tile([128, 512], F32, tag="psm")
                    nc.tensor.matmul(
                        P[:, :padg],
                        fA[:, idx * 128 : (idx + 1) * 128],
                        fB[:, cs : cs + padg],
                        start=True,
                        stop=True,
                    )
                    nc.vector.tensor_tensor(
                        P[:, off : off + 128], P[:, off : off + 128], seld[:], op=ALU.add
                    )
                    nc.vector.tensor_reduce(
                        mpart[:, idx : idx + 1], P[:, :padg], axis=AX.X, op=ALU.min
                    )

            # ---- dist_pos (off critical path): || an - bn || ----
            bno = wpool.tile([128, rt * D], F32, tag="bno")
            nc.vector.tensor_tensor(
                bno[:, :].rearrange("p (t d) -> p t d", d=D),
                tB3,
                rB[:, :].broadcast_to([128, rt, D]),
                op=ALU.mult,
            )
            dd = wpool.tile([128, rt * D], F32, tag="dd")
            nc.vector.tensor_tensor(dd[:], an[:], bno[:], op=ALU.subtract)
            sqd = wpool.tile([128, rt * D], F32, tag="sqd")
            nc.scalar.activation(sqd[:], dd[:], AF.Square)
            dp2 = wpool.tile([128, rt], F32, tag="dp2")
            nc.vector.tensor_reduce(
                dp2[:], sqd[:, :].rearrange("p (t d) -> p t d", d=D), axis=AX.X, op=ALU.add
            )
            dpos = wpool.tile([128, rt], F32, tag="dpos")
            nc.scalar.activation(dpos[:], dp2[:], AF.Sqrt)

            # ---- finalize ----
            mc = wpool.tile([128, rt], F32, tag="mc")
            nc.vector.tensor_scalar(mc[:], mpart[:], 0.0, None, op0=ALU.max)
            sn = wpool.tile([128, rt], F32, tag="sn")
            nc.scalar.activation(sn[:], mc[:], AF.Sqrt)
            valid = wpool.tile([128, rt], F32, tag="valid")
            nc.vector.tensor_scalar(valid[:], mpart[:], TH, None, op0=ALU.is_lt)
            dn = wpool.tile([128, rt], F32, tag="dn")
            nc.vector.tensor_tensor(dn[:], sn[:], valid[:], op=ALU.mult)
            pre = wpool.tile([128, rt], F32, tag="pre")
            nc.vector.tensor_tensor(pre[:], dpos[:], dn[:], op=ALU.subtract)
            losses = wpool.tile([128, rt], F32, tag="losses")
            nc.scalar.activation(losses[:], pre[:], AF.Relu, bias=MARGIN)
            nc.sync.dma_start(out=out[:, :], in_=losses[:])

    _split_drain_waits(nc)
    return nc


_NC_CACHE = None
_NC_SORTED_CACHE = {}


def _get_nc():
    global _NC_CACHE
    if _NC_CACHE is None:
        _NC_CACHE = _build_nc()
    return _NC_CACHE


def _get_nc_sorted(gpc, padg):
    key = (gpc, padg)
    if key not in _NC_SORTED_CACHE:
        _NC_SORTED_CACHE[key] = _build_nc_sorted(gpc, padg)
    return _NC_SORTED_CACHE[key]


def _tile_tp(x):
    """[R, 32] rows -> [128, (R/128)*32] with row t*128+p on partition p."""
    r = x.shape[0]
    return (
        np.ascontiguousarray(
            x.reshape(r // 128, 128, D).transpose(1, 0, 2).reshape(128, (r // 128) * D)
        )
    )


def _kernel_sorted(A, B, lab):
    counts = np.bincount(lab, minlength=G)
    gn = len(counts)
    gpc = -(-gn // NCORES)
    padg = max(128, -(-int(counts.max()) // 128) * 128)
    if padg > 512:
        return None  # degenerate label distribution: fall back to full kernel
    rmax = gpc * padg
    rt = rmax // 128

    order = np.argsort(lab, kind="stable")
    starts = np.concatenate([[0], np.cumsum(counts)])

    src = np.full((NCORES, rmax), -1, np.int64)
    for g in range(gn):
        c, gl = divmod(g, gpc)
        n = int(counts[g])
        src[c, gl * padg : gl * padg + n] = order[starts[g] : starts[g] + n]

    in_maps = []
    for c in range(NCORES):
        idx = src[c]
        real = idx >= 0
        a_rows = np.ones((rmax, D), np.float32)
        b_rows = np.ones((rmax, D), np.float32)
        a_rows[real] = A[idx[real]]
        b_rows[real] = B[idx[real]]
        cv = np.ones((2, rmax), np.float32)
        cv[1] = np.where(real, 2.0, 2.0 + BIG)
        in_maps.append(
            {
                "a": _tile_tp(a_rows),
                "b": _tile_tp(b_rows),
                "cv": np.ascontiguousarray(cv),
            }
        )

    global _last_in_maps, _last_nc
    _last_in_maps = in_maps
    nc = _get_nc_sorted(gpc, padg)
    _last_nc = nc
    res = run_bass_kernel_spmd(nc, in_maps, list(range(NCORES)))
    total = 0.0
    for c in range(NCORES):
        lo = res.results[c]["losses"]  # [128, rt]; [p, t] = loss of local row t*128+p
        flat = lo.T.reshape(rmax)
        real = src[c] >= 0
        total += float(flat[real].sum(dtype=np.float64))
    return np.float32(total / N)


def kernel(A=None, B=None, labels=None, **_unused):
    import os

    A = np.asarray(A, dtype=np.float32)
    B = np.asarray(B, dtype=np.float32)
    lab = np.asarray(labels).astype(np.int64)
    lab = lab - lab.min() if lab.min() < 0 else lab

    if not os.environ.get("KERNEL_FORCE_FULL"):
        out = _kernel_sorted(A, B, lab.astype(np.int32))
        if out is not None:
            return out

    eye = np.arange(G, dtype=np.int32)
    in_maps = []
    for c in range(NCORES):
        rows = slice(c * RPC, (c + 1) * RPC)
        a_c = _tile_tp(A[rows])
        b_rot = np.roll(B, -c * RPC, axis=0)
        lab_rot = np.roll(lab, -c * RPC)
        b_c = _tile_tp(b_rot)
        oha = np.concatenate(
            [
                np.ones((1, RPC), np.float32),
                (-BIG) * (lab[rows][None, :] == eye[:, None]).astype(np.float32),
            ]
        )
        ohb = np.concatenate(
            [
                np.full((1, N), 2.0 + BIG, np.float32),
                (lab_rot[None, :] == eye[:, None]).astype(np.float32),
            ]
        )
        in_maps.append(
            {
                "a": a_c,
                "b": b_c,
                "oha": np.ascontiguousarray(oha),
                "ohb": np.ascontiguousarray(ohb),
            }
        )

    global _last_in_maps, _last_nc
    _last_in_maps = in_maps
    nc = _get_nc()
    _last_nc = nc
    res = run_bass_kernel_spmd(nc, in_maps, list(range(NCORES)))
    total = 0.0
    for c in range(NCORES):
        lo = res.results[c]["losses"]  # [128, RT]; [p, r] = loss of row r*128+p
        total += float(lo.sum(dtype=np.float64))
    return np.float32(total / N)



# revision 11
# speedup vs baseline: 1.1157x; 1.1157x over previous
"""Grouped triplet loss on 8 trn2 NeuronCores.

Strategy (data-parallel over A rows, hint-compliant):
  - Each core takes a 1024-row block of A, full B (column-rotated so the
    diagonal of the distance matrix lands at core-independent positions).
  - L2 normalization of A-block and B on device.
  - One fused matmul per (row-tile, col-chunk) computes the *masked* squared
    distance directly in PSUM via extended feature vectors:
        F_A = [ a_i (32) | 1 | -BIG*onehot(label_i) (32) ]   (K = 65)
        F_B = [ -2*b_j   | 2+BIG |      onehot(label_j)  ]
    so PSUM = 2 - 2*a.b + BIG*(1 - same_group).
  - A tiny bf16 identity matmul accumulates +BIG on the diagonal (self-pair
    exclusion).
  - DVE min-reduces PSUM (4 banks per op); rows with min >= TH had no valid
    negative -> dist_neg = 0 (matches torch "skip groups of size < 2").
  - losses = relu(dist_pos - dist_neg + margin); host averages.

Host-side work is limited to sharding/layout: slicing, row-rotation, (t p)
tiling, and one-hot encoding of the integer labels. All float math happens
on device.
"""

import numpy as np

import concourse.bass as bass
import concourse.mybir as mybir
from concourse.tile import TileContext
from concourse.bass_utils import run_bass_kernel_spmd

N, D, G = 8192, 32, 32
NCORES = 8
RPC = N // NCORES      # rows per core = 1024
RT = RPC // 128        # row tiles per core = 8
CT = N // 128          # column tiles = 64
NCHUNK = N // 512      # matmul column chunks = 16
BIG = 64.0
TH = 32.0
MARGIN = 1.0

F32 = mybir.dt.float32
BF16 = mybir.dt.bfloat16
AF = mybir.ActivationFunctionType
ALU = mybir.AluOpType
AX = mybir.AxisListType

MM_DT = mybir.dt.float32r  # matmul feature dtype (float32 | float32r)

_MAX_DRAIN_WAITS = 1

import os as _os
_V4_BN_DVE = bool(_os.environ.get("V4_BN_DVE"))
_V4_AB_DVE = bool(_os.environ.get("V4_AB_DVE"))
_V4_DEBUG = bool(_os.environ.get("V4_DEBUG"))


def _split_drain_waits(nc):
    """This container's walrus rejects any instruction with >1 sem-wait.
    Hoist excess waits onto preceding same-engine single-wait Drains."""
    nsplit = 0
    for f in nc.m.functions:
        for bb in f.blocks:
            new_insts = []
            for inst in bb.instructions:
                si = inst.sync_info
                waits = list(si.on_wait) if si and si.on_wait else []
                if len(waits) > _MAX_DRAIN_WAITS:
                    extra, keep = waits[:-_MAX_DRAIN_WAITS], waits[-_MAX_DRAIN_WAITS:]
                    for w in extra:
                        d = mybir.InstDrain(
                            name=f"{inst.name}-swsplit{nsplit}",
                            engine=inst.engine,
                            ins=[],
                            outs=[],
                            sync_info=mybir.SyncInfo(on_wait=[w], on_update=[]),
                        )
                        nsplit += 1
                        nc.register_instruction(d, overwrite=True)
                        new_insts.append(d)
                    si.on_wait = keep
                new_insts.append(inst)
            bb.instructions[:] = new_insts


def _build_nc():
    import ml_dtypes

    nc = bass.Bass()

    a_in = nc.dram_tensor("a", [128, RT * D], F32, kind="ExternalInput")
    b_in = nc.dram_tensor("b", [128, CT * D], F32, kind="ExternalInput")
    # row 0: constant feature (1 for A, 2+BIG for B); rows 1..32: one-hot
    oha_in = nc.dram_tensor("oha", [G + 1, RPC], MM_DT, kind="ExternalInput")
    ohb_in = nc.dram_tensor("ohb", [G + 1, N], MM_DT, kind="ExternalInput")
    out = nc.dram_tensor("losses", [128, RT], F32, kind="ExternalOutput")

    ident_np = np.eye(128, dtype=np.float32)
    sel_np = np.zeros((128, 1024), dtype=np.float32)
    sel_np[np.arange(128), 512 + np.arange(128)] = 1.0
    bigi_np = (BIG * np.eye(128)).astype(ml_dtypes.bfloat16)
    ident_d = nc.inline_tensor(ident_np, name="identc")
    sel_d = nc.inline_tensor(sel_np.astype(ml_dtypes.bfloat16), name="selc")
    bigi_d = nc.inline_tensor(bigi_np, name="bigic")

    with TileContext(nc) as tc:
        with (
            tc.tile_pool(name="const", bufs=1) as cpool,
            tc.tile_pool(name="work", bufs=1) as wpool,
            tc.tile_pool(name="ps", bufs=2, space="PSUM") as pspool,
        ):
            # ---- constants -------------------------------------------------
            ident = cpool.tile([128, 128], F32, tag="ident")
            nc.sync.dma_start(out=ident[:], in_=ident_d[:, :])
            sel = cpool.tile([128, 1024], BF16, tag="sel")
            nc.sync.dma_start(out=sel[:], in_=sel_d[:, :])
            bigi = cpool.tile([128, 128], BF16, tag="bigi")
            nc.sync.dma_start(out=bigi[:], in_=bigi_d[:, :])

            # ---- raw loads -------------------------------------------------
            tA = wpool.tile([128, RT * D], F32, tag="tA")
            nc.sync.dma_start(out=tA[:], in_=a_in[:, :])
            tB = wpool.tile([128, CT * D], F32, tag="tB")
            # split into 2 DMAs to use more queues
            nc.sync.dma_start(out=tB[:, : CT * D // 2], in_=b_in[:, : CT * D // 2])
            nc.sync.dma_start(out=tB[:, CT * D // 2 :], in_=b_in[:, CT * D // 2 :])

            fA = cpool.tile([G + 33, RPC], MM_DT, tag="fA")
            fB = cpool.tile([G + 33, N], MM_DT, tag="fB")
            nc.sync.dma_start(out=fA[32:65, :], in_=oha_in[:, :])
            nc.sync.dma_start(out=fB[32:65, : N // 2], in_=ohb_in[:, : N // 2])
            nc.sync.dma_start(out=fB[32:65, N // 2 :], in_=ohb_in[:, N // 2 :])

            # ---- normalize A block ----------------------------------------
            tA3 = tA[:, :].rearrange("p (t d) -> p t d", d=D)
            sqA = wpool.tile([128, RT * D], F32, tag="sqA")
            nc.scalar.activation(sqA[:], tA[:], AF.Square)
            ssA = wpool.tile([128, RT], F32, tag="ssA")
            nc.vector.tensor_reduce(
                ssA[:], sqA[:, :].rearrange("p (t d) -> p t d", d=D), axis=AX.X, op=ALU.add
            )
            nA = wpool.tile([128, RT], F32, tag="nA")
            nc.scalar.activation(nA[:], ssA[:], AF.Sqrt)
            rA = wpool.tile([128, RT], F32, tag="rA")
            nc.vector.reciprocal(rA[:], nA[:])
            an = wpool.tile([128, RT * D], F32, tag="an")
            an3 = an[:, :].rearrange("p (t d) -> p t d", d=D)
            nc.vector.tensor_tensor(
                an3, tA3, rA[:, :].broadcast_to([128, RT, D]), op=ALU.mult
            )

            # ---- normalize B (scaled by -2 for features) -------------------
            tB3 = tB[:, :].rearrange("p (t d) -> p t d", d=D)
            sqB = wpool.tile([128, CT * D], F32, tag="sqB")
            nc.scalar.activation(sqB[:], tB[:], AF.Square)
            ssB = wpool.tile([128, CT], F32, tag="ssB")
            nc.vector.tensor_reduce(
                ssB[:], sqB[:, :].rearrange("p (t d) -> p t d", d=D), axis=AX.X, op=ALU.add
            )
            nB = wpool.tile([128, CT], F32, tag="nB")
            nc.scalar.activation(nB[:], ssB[:], AF.Sqrt)
            rB = wpool.tile([128, CT], F32, tag="rB")
            nc.vector.reciprocal(rB[:], nB[:])
            rBm2 = wpool.tile([128, CT], F32, tag="rBm2")
            nc.vector.tensor_scalar(rBm2[:], rB[:], -2.0, None, op0=ALU.mult)
            bn2 = wpool.tile([128, CT * D], F32, tag="bn2")
            bn23 = bn2[:, :].rearrange("p (t d) -> p t d", d=D)
            nc.vector.tensor_tensor(
                bn23, tB3, rBm2[:, :].broadcast_to([128, CT, D]), op=ALU.mult
            )

            # ---- transpose an -> fA[0:32, :] ------------------------------
            psA = pspool.tile([32, RPC], F32, tag="ps")
            for r in range(RT):
                nc.tensor.transpose(psA[:, r * 128 : (r + 1) * 128], an3[:, r, :], ident[:])
            nc.scalar.copy(fA[0:32, :], psA[:, :])

            # ---- transpose bn2 -> fB[0:32, :] ------------------------------
            for grp in range(CT // 16):
                psB = pspool.tile([32, 16 * 128], F32, tag="ps")
                for k in range(16):
                    t = grp * 16 + k
                    nc.tensor.transpose(
                        psB[:, k * 128 : (k + 1) * 128], bn23[:, t, :], ident[:]
                    )
                nc.scalar.copy(fB[0:32, grp * 2048 : (grp + 1) * 2048], psB[:, :])

            # ---- dist_pos for own rows (first RT tiles of rotated B) ------
            bno = wpool.tile([128, RT * D], F32, tag="bno")
            bno3 = bno[:, :].rearrange("p (t d) -> p t d", d=D)
            nc.vector.tensor_tensor(
                bno3, tB3[:, 0:RT, :], rB[:, 0:RT].broadcast_to([128, RT, D]), op=ALU.mult
            )
            dd = wpool.tile([128, RT * D], F32, tag="dd")
            nc.vector.tensor_tensor(dd[:], an[:], bno[:], op=ALU.subtract)
            sqd = wpool.tile([128, RT * D], F32, tag="sqd")
            nc.scalar.activation(sqd[:], dd[:], AF.Square)
            dp2 = wpool.tile([128, RT], F32, tag="dp2")
            nc.vector.tensor_reduce(
                dp2[:], sqd[:, :].rearrange("p (t d) -> p t d", d=D), axis=AX.X, op=ALU.add
            )
            dpos = wpool.tile([128, RT], F32, tag="dpos")
            nc.scalar.activation(dpos[:], dp2[:], AF.Sqrt)

            # ---- main loop: fused matmul + masked min ----------------------
            mpart = wpool.tile([128, RT * 4], F32, tag="mpart")
            for r in range(RT):
                lhsT = fA[:, r * 128 : (r + 1) * 128]
                for q in range(4):
                    P4 = pspool.tile([128, 2048], F32, tag="ps")
                    for j in range(4):
                        c = q * 4 + j
                        is_diag = q == 0 and j == r // 4
                        nc.tensor.matmul(
                            P4[:, j * 512 : (j + 1) * 512],
                            lhsT,
                            fB[:, c * 512 : (c + 1) * 512],
                            start=True,
                            stop=not is_diag,
                        )
                        if is_diag:
                            off = (r % 4) * 128
                            nc.tensor.matmul(
                                P4[:, j * 512 : (j + 1) * 512],
                                bigi[:],
                                sel[:, 512 - off : 1024 - off],
                                start=False,
                                stop=True,
                            )
                    nc.vector.tensor_reduce(
                        mpart[:, r * 4 + q : r * 4 + q + 1],
                        P4[:, :].rearrange("p (f c) -> p f c", c=512),
                        axis=AX.XY,
                        op=ALU.min,
                    )

            # ---- finalize --------------------------------------------------
            m = wpool.tile([128, RT], F32, tag="m")
            nc.vector.tensor_reduce(
                m[:], mpart[:, :].rearrange("p (r q) -> p r q", q=4), axis=AX.X, op=ALU.min
            )
            mc = wpool.tile([128, RT], F32, tag="mc")
            nc.vector.tensor_scalar(mc[:], m[:], 0.0, None, op0=ALU.max)
            sn = wpool.tile([128, RT], F32, tag="sn")
            nc.scalar.activation(sn[:], mc[:], AF.Sqrt)
            valid = wpool.tile([128, RT], F32, tag="valid")
            nc.vector.tensor_scalar(valid[:], m[:], TH, None, op0=ALU.is_lt)
            dn = wpool.tile([128, RT], F32, tag="dn")
            nc.vector.tensor_tensor(dn[:], sn[:], valid[:], op=ALU.mult)
            pre = wpool.tile([128, RT], F32, tag="pre")
            nc.vector.tensor_tensor(pre[:], dpos[:], dn[:], op=ALU.subtract)
            losses = wpool.tile([128, RT], F32, tag="losses")
            nc.scalar.activation(losses[:], pre[:], AF.Relu, bias=MARGIN)
            nc.sync.dma_start(out=out[:, :], in_=losses[:])

    _split_drain_waits(nc)
    return nc


def _build_nc_sorted(gpc, padg):
    """Group-sorted variant: each core gets `gpc` whole groups, each padded to
    `padg` rows/cols. Only within-group blocks are computed (the masked min
    never needs cross-group pairs). Columns = the core's own rows, so the
    self-pair diagonal sits at block-local positions; it is excluded by an
    in-place +BIG*I add on the 128-wide diagonal slab before the min-reduce.
    Padded columns carry constant-feature 2+BIG -> always excluded.

    Structured as a per-group pipeline: transpose -> feature copy -> matmul ->
    diag add -> min reduce, so PE/ACT/DVE overlap across groups. The B chain
    is emitted first (it gates the feature build); dist_pos is emitted last
    (only needed by the finalize stage)."""
    assert padg <= 512 and padg % 128 == 0
    rmax = gpc * padg          # rows (and cols) per core
    rt = rmax // 128           # 128-row tiles per core
    tpg = padg // 128          # row tiles per group

    nc = bass.Bass()
    a_in = nc.dram_tensor("a", [128, rt * D], F32, kind="ExternalInput")
    b_in = nc.dram_tensor("b", [128, rt * D], F32, kind="ExternalInput")
    cv_in = nc.dram_tensor("cv", [2, rmax], MM_DT, kind="ExternalInput")
    out = nc.dram_tensor("losses", [128, rt], F32, kind="ExternalOutput")
    dbg = None
    if _V4_DEBUG:
        dbg = {
            "mval": nc.dram_tensor("d_mval", [128, rt], F32, kind="ExternalOutput"),
            "rA": nc.dram_tensor("d_rA", [128, rt], F32, kind="ExternalOutput"),
            "rB": nc.dram_tensor("d_rB", [128, rt], F32, kind="ExternalOutput"),
            "abr": nc.dram_tensor("d_abr", [128, rt], F32, kind="ExternalOutput"),
            "dp": nc.dram_tensor("d_dp", [128, rt], F32, kind="ExternalOutput"),
            "dn": nc.dram_tensor("d_dn", [128, rt], F32, kind="ExternalOutput"),
            "fb": nc.dram_tensor("d_fb", [33, rt * 128], F32, kind="ExternalOutput"),
        }

    ident_np = np.eye(128, dtype=np.float32)
    seld_np = (BIG * np.eye(128)).astype(np.float32)
    ident_d = nc.inline_tensor(ident_np, name="identc")
    seld_d = nc.inline_tensor(seld_np, name="seldc")

    half = rt * D // 2

    with TileContext(nc) as tc:
        with (
            tc.tile_pool(name="const", bufs=1) as cpool,
            tc.tile_pool(name="work", bufs=1) as wpool,
            tc.tile_pool(name="pst", bufs=2, space="PSUM") as pstp,
            tc.tile_pool(name="psm", bufs=4, space="PSUM") as psmp,
        ):
            # input DMAs first, spread across otherwise-idle engine queues
            tB = wpool.tile([128, rt * D], F32, tag="tB")
            nc.sync.dma_start(out=tB[:, :half], in_=b_in[:, :half])
            nc.sync.dma_start(out=tB[:, half:], in_=b_in[:, half:])
            tA = wpool.tile([128, rt * D], F32, tag="tA")
            nc.gpsimd.dma_start(out=tA[:, :half], in_=a_in[:, :half])
            nc.gpsimd.dma_start(out=tA[:, half:], in_=a_in[:, half:])

            ident = cpool.tile([128, 128], F32, tag="ident")
            nc.scalar.dma_start(out=ident[:], in_=ident_d[:, :])
            seld = cpool.tile([128, 128], F32, tag="seld")
            nc.scalar.dma_start(out=seld[:], in_=seld_d[:, :])

            fA = cpool.tile([33, rmax], MM_DT, tag="fA")
            fB = cpool.tile([33, rmax], MM_DT, tag="fB")
            nc.scalar.dma_start(out=fB[32:33, :], in_=cv_in[1:2, :])
            nc.scalar.dma_start(out=fA[32:33, :], in_=cv_in[0:1, :])

            # fire the ACT table load immediately (contents irrelevant)
            warmup_act = wpool.tile([128, 8], F32, tag="warmup_act")
            nc.scalar.activation(warmup_act[:], warmup_act[:], AF.Square)

            # ---- B chain (critical: gates the feature build) ----
            tB3 = tB[:, :].rearrange("p (t d) -> p t d", d=D)
            sqB = wpool.tile([128, rt * D], F32, tag="sqB")
            nc.scalar.activation(sqB[:, :half], tB[:, :half], AF.Square)
            nc.scalar.activation(sqB[:, half:], tB[:, half:], AF.Square)
            ssB = wpool.tile([128, rt], F32, tag="ssB")
            nc.vector.tensor_reduce(
                ssB[:], sqB[:, :].rearrange("p (t d) -> p t d", d=D), axis=AX.X, op=ALU.add
            )
            nB = wpool.tile([128, rt], F32, tag="nB")
            nc.scalar.activation(nB[:], ssB[:], AF.Sqrt)
            rB = wpool.tile([128, rt], F32, tag="rB")
            nc.vector.reciprocal(rB[:], nB[:])
            rBm2 = wpool.tile([128, rt], F32, tag="rBm2")
            nc.vector.tensor_scalar(rBm2[:], rB[:], -2.0, None, op0=ALU.mult)
            bn2 = wpool.tile([128, rt * D], F32, tag="bn2")
            bn23 = bn2[:, :].rearrange("p (t d) -> p t d", d=D)
            nc.vector.tensor_tensor(
                bn23, tB3, rBm2[:, :].broadcast_to([128, rt, D]), op=ALU.mult
            )

            # ---- A chain ----
            tA3 = tA[:, :].rearrange("p (t d) -> p t d", d=D)
            sqA = wpool.tile([128, rt * D], F32, tag="sqA")
            nc.scalar.activation(sqA[:, :half], tA[:, :half], AF.Square)
            nc.scalar.activation(sqA[:, half:], tA[:, half:], AF.Square)
            ssA = wpool.tile([128, rt], F32, tag="ssA")
            nc.vector.tensor_reduce(
                ssA[:], sqA[:, :].rearrange("p (t d) -> p t d", d=D), axis=AX.X, op=ALU.add
            )
            nA = wpool.tile([128, rt], F32, tag="nA")
            nc.scalar.activation(nA[:], ssA[:], AF.Sqrt)
            rA = wpool.tile([128, rt], F32, tag="rA")
            nc.vector.reciprocal(rA[:], nA[:])
            an = wpool.tile([128, rt * D], F32, tag="an")
            an3 = an[:, :].rearrange("p (t d) -> p t d", d=D)
            nc.vector.tensor_tensor(
                an3, tA3, rA[:, :].broadcast_to([128, rt, D]), op=ALU.mult
            )

            # ---- PE warm-up: dummy transposes keyed to sqB so the HAM
            # clock-gate opens before the real transposes/matmuls arrive ----
            for w in range(16):
                pw = psmp.tile([128, 512], F32, tag="psm")
                nc.tensor.transpose(pw[:, 0:128], sqB[:, 0:128], ident[:])

            # ---- per-group pipeline ----
            mpart = wpool.tile([128, rt], F32, tag="mpart")
            for gl in range(gpc):
                base = gl * tpg
                cs = gl * padg
                psB = pstp.tile([32, padg], F32, tag="pstB")
                for r in range(tpg):
                    nc.tensor.transpose(
                        psB[:, r * 128 : (r + 1) * 128], bn23[:, base + r, :], ident[:]
                    )
                nc.scalar.copy(fB[0:32, cs : cs + padg], psB[:, :])
                psA = pstp.tile([32, padg], F32, tag="pstA")
                for r in range(tpg):
                    nc.tensor.transpose(
                        psA[:, r * 128 : (r + 1) * 128], an3[:, base + r, :], ident[:]
                    )
                nc.scalar.copy(fA[0:32, cs : cs + padg], psA[:, :])
                for r in range(tpg):
                    idx = base + r
                    off = r * 128
                    P = psmp.tile([128, 512], F32, tag="psm")
                    nc.tensor.matmul(
                        P[:, :padg],
                        fA[:, idx * 128 : (idx + 1) * 128],
                        fB[:, cs : cs + padg],
                        start=True,
                        stop=True,
                    )
                    nc.vector.tensor_tensor(
                        P[:, off : off + 128], P[:, off : off + 128], seld[:], op=ALU.add
                    )
                    nc.vector.tensor_reduce(
                        mpart[:, idx : idx + 1], P[:, :padg], axis=AX.X, op=ALU.min
                    )

            # ---- dist_pos (off critical path): || an - bn || ----
            bno = wpool.tile([128, rt * D], F32, tag="bno")
            nc.vector.tensor_tensor(
                bno[:, :].rearrange("p (t d) -> p t d", d=D),
                tB3,
                rB[:, :].broadcast_to([128, rt, D]),
                op=ALU.mult,
            )
            dd = wpool.tile([128, rt * D], F32, tag="dd")
            nc.vector.tensor_tensor(dd[:], an[:], bno[:], op=ALU.subtract)
            sqd = wpool.tile([128, rt * D], F32, tag="sqd")
            nc.scalar.activation(sqd[:], dd[:], AF.Square)
            dp2 = wpool.tile([128, rt], F32, tag="dp2")
            nc.vector.tensor_reduce(
                dp2[:], sqd[:, :].rearrange("p (t d) -> p t d", d=D), axis=AX.X, op=ALU.add
            )
            dpos = wpool.tile([128, rt], F32, tag="dpos")
            nc.scalar.activation(dpos[:], dp2[:], AF.Sqrt)

            # ---- finalize ----
            mc = wpool.tile([128, rt], F32, tag="mc")
            nc.vector.tensor_scalar(mc[:], mpart[:], 0.0, None, op0=ALU.max)
            sn = wpool.tile([128, rt], F32, tag="sn")
            nc.scalar.activation(sn[:], mc[:], AF.Sqrt)
            valid = wpool.tile([128, rt], F32, tag="valid")
            nc.vector.tensor_scalar(valid[:], mpart[:], TH, None, op0=ALU.is_lt)
            dn = wpool.tile([128, rt], F32, tag="dn")
            nc.vector.tensor_tensor(dn[:], sn[:], valid[:], op=ALU.mult)
            pre = wpool.tile([128, rt], F32, tag="pre")
            nc.vector.tensor_tensor(pre[:], dpos[:], dn[:], op=ALU.subtract)
            losses = wpool.tile([128, rt], F32, tag="losses")
            nc.scalar.activation(losses[:], pre[:], AF.Relu, bias=MARGIN)
            nc.sync.dma_start(out=out[:, :], in_=losses[:])
            if dbg is not None:
                nc.sync.dma_start(out=dbg["mval"][:, :], in_=mval[:])
                nc.sync.dma_start(out=dbg["rA"][:, :], in_=rA[:])
                nc.sync.dma_start(out=dbg["rB"][:, :], in_=rB[:])
                nc.sync.dma_start(out=dbg["abr"][:, :], in_=abr[:])
                nc.sync.dma_start(out=dbg["dp"][:, :], in_=dp[:])
                nc.sync.dma_start(out=dbg["dn"][:, :], in_=dn[:])
                fbf = wpool.tile([33, RMAX], F32, tag="fbf")
                nc.scalar.copy(fbf[:], fb[:, :])
                nc.sync.dma_start(out=dbg["fb"][:, :], in_=fbf[:])

    _split_drain_waits(nc)
    return nc


def _build_nc_v4(slots, use_valid):
    """v4: column-tight group-sorted kernel, max-cos formulation.

    slots: tuple of (tiles, padc) per slot (same structure on all 8 cores).
    Layout per core: rt = sum(tiles) 128-row tiles; slot s occupies row tiles
    [base_s, base_s+tiles_s) and fb columns [128*base_s, ...).

    Math (per local row i of group g, columns j over g's padded block):
      PSUM_ij = a_i . b^_j + cv_j  (+ -BIG on self col via diag matmul)
      cv_j = 0 for real cols, -BIG for padding  -> masked entries < -TH
      mval_i = max_j PSUM_ij ;  cos_neg = min(mval*rA, 1)
      dist_neg = sqrt(2 - 2*cos_neg) ; dist_pos = sqrt(2 - 2*min(ab*rA*rB, 1))
      loss = relu(dist_pos - dist_neg + 1)     (host sums real rows)

    A is used RAW in the matmul (features [A^T; 1], host-transposed, bf16);
    the 1/|a| scaling is applied after the max (order-preserving), so no
    A-side transposes or pre-normalization sit on the critical path.
    """
    rt = sum(t for t, _ in slots)
    bases = []
    b0 = 0
    for t, _ in slots:
        bases.append(b0)
        b0 += t
    W = rt * D           # row-major feature width per partition
    RMAX = rt * 128      # fa/fb columns
    half_t = (rt + 1) // 2
    half = half_t * D

    nc = bass.Bass()
    a_in = nc.dram_tensor("a", [128, W], F32, kind="ExternalInput")
    b_in = nc.dram_tensor("b", [128, W], F32, kind="ExternalInput")
    fa_in = nc.dram_tensor("fa", [33, RMAX], BF16, kind="ExternalInput")
    cv_in = nc.dram_tensor("cv", [1, RMAX], BF16, kind="ExternalInput")
    out = nc.dram_tensor("losses", [128, rt], F32, kind="ExternalOutput")
    dbg = None
    if _V4_DEBUG:
        dbg = {
            "mval": nc.dram_tensor("d_mval", [128, rt], F32, kind="ExternalOutput"),
            "rA": nc.dram_tensor("d_rA", [128, rt], F32, kind="ExternalOutput"),
            "rB": nc.dram_tensor("d_rB", [128, rt], F32, kind="ExternalOutput"),
            "abr": nc.dram_tensor("d_abr", [128, rt], F32, kind="ExternalOutput"),
            "dp": nc.dram_tensor("d_dp", [128, rt], F32, kind="ExternalOutput"),
            "dn": nc.dram_tensor("d_dn", [128, rt], F32, kind="ExternalOutput"),
            "fb": nc.dram_tensor("d_fb", [33, rt * 128], F32, kind="ExternalOutput"),
        }

    with TileContext(nc) as tc:
        with (
            tc.tile_pool(name="const", bufs=1) as cpool,
            tc.tile_pool(name="work", bufs=1) as wpool,
            tc.tile_pool(name="pst", bufs=2, space="PSUM") as pstp,
            tc.tile_pool(name="psm", bufs=2, space="PSUM") as psmp,
        ):
            # ---- input DMAs (sync + gpsimd queues; scalar kept free for ACT)
            tB = wpool.tile([128, W], F32, tag="tB")
            nc.sync.dma_start(out=tB[:, :half], in_=b_in[:, :half])
            fa = cpool.tile([33, RMAX], BF16, tag="fa")
            nc.sync.dma_start(out=fa[:, :], in_=fa_in[:, :])
            fb = cpool.tile([33, RMAX], BF16, tag="fb")
            nc.sync.dma_start(out=fb[32:33, :], in_=cv_in[:, :])
            tA = wpool.tile([128, W], F32, tag="tA")

            # ---- ACT warm-up: trigger the activation table load immediately
            warm = wpool.tile([128, 8], F32, tag="warm")
            nc.scalar.activation(warm[:], warm[:], AF.Square)

            # ---- gpsimd: DMAs interleaved with constant builds
            wz = cpool.tile([32, 512], BF16, tag="wz")
            nc.gpsimd.memset(wz[:, :], 0.0)
            nc.gpsimd.dma_start(out=tB[:, half:], in_=b_in[:, half:])
            # identwide[p, z] = 1 iff z == p + 384  (sliding window for diag mms)
            IW = 384 + 512
            identw = cpool.tile([128, IW], BF16, tag="identw")
            nc.gpsimd.memset(identw[:, :], 0.0)
            nc.gpsimd.affine_select(
                identw[:, 384:512], identw[:, 384:512], pattern=[[1, 128]],
                compare_op=ALU.not_equal, fill=1.0, base=0, channel_multiplier=-1,
            )
            nc.gpsimd.dma_start(out=tA[:, :half], in_=a_in[:, :half])
            nc.gpsimd.dma_start(out=tA[:, half:], in_=a_in[:, half:])
            nbig = cpool.tile([128, 128], BF16, tag="nbig")
            nc.gpsimd.memset(nbig[:, :], 0.0)
            nc.gpsimd.affine_select(
                nbig[:, :], nbig[:, :], pattern=[[1, 128]],
                compare_op=ALU.not_equal, fill=-BIG, base=0, channel_multiplier=-1,
            )
            bias2 = cpool.tile([128, 1], F32, tag="bias2")
            nc.gpsimd.memset(bias2[:, :], 2.0)

            # ---- PE warm-up matmuls (ramp the p-state during DMA/chain phase)
            for wmi in range(6):
                psw = psmp.tile([128, 1536], F32, tag="P")
                nc.tensor.matmul(
                    psw[:, 0:384], wz[:, 0:128], wz[:, 128:512],
                    start=True, stop=True,
                )

            # ---- B chain: |b| -> rB -> bn = b * rB (bf16)
            tB3 = tB[:, :].rearrange("p (t d) -> p t d", d=D)
            sqB = wpool.tile([128, W], F32, tag="sqB")
            nc.scalar.activation(sqB[:, :half], tB[:, :half], AF.Square)
            nc.scalar.activation(sqB[:, half:], tB[:, half:], AF.Square)
            ssB = wpool.tile([128, rt], F32, tag="ssB")
            nc.vector.tensor_reduce(
                ssB[:], sqB[:, :].rearrange("p (t d) -> p t d", d=D), axis=AX.X, op=ALU.add
            )
            nB = wpool.tile([128, rt], F32, tag="nB")
            nc.scalar.activation(nB[:], ssB[:], AF.Sqrt)
            rB = wpool.tile([128, rt], F32, tag="rB")
            nc.vector.reciprocal(rB[:], nB[:])
            bn = wpool.tile([128, W], BF16, tag="bn")
            bn3 = bn[:, :].rearrange("p (t d) -> p t d", d=D)
            _BN_ENG = nc.vector if _V4_BN_DVE else nc.gpsimd
            _BN_ENG.tensor_tensor(
                bn3, tB3, rB[:, :].broadcast_to([128, rt, D]), op=ALU.mult
            )

            # ---- ab = sum_d a*b (raw), on gpsimd+DVE, off critical path
            ab = wpool.tile([128, W], F32, tag="ab")
            (nc.vector if _V4_AB_DVE else nc.gpsimd).tensor_tensor(ab[:], tA[:], tB[:], op=ALU.mult)

            # ---- per-slot pipeline: transpose -> copy -> matmuls(+diag) -> reduce
            mval = wpool.tile([128, rt], F32, tag="mval")
            red_jobs = []
            for si, (tiles, padc) in enumerate(slots):
                base = bases[si]
                cs = base * 128
                psT = pstp.tile([32, tiles * 128], BF16, tag="psT")
                for k in range(tiles):
                    nc.tensor.transpose(
                        psT[:, k * 128 : (k + 1) * 128], bn3[:, base + k, :], identw[:, 384:512]
                    )
                nc.scalar.copy(fb[0:32, cs : cs + tiles * 128], psT[:, :])
                P = psmp.tile([128, 1536], F32, tag="P")
                for k in range(tiles):
                    nc.tensor.matmul(
                        P[:, k * 512 : k * 512 + padc],
                        fa[:, (base + k) * 128 : (base + k + 1) * 128],
                        fb[:, cs : cs + padc],
                        start=True, stop=False,
                    )
                    nc.tensor.matmul(
                        P[:, k * 512 : k * 512 + padc],
                        nbig[:],
                        identw[:, 384 - 128 * k : 384 - 128 * k + padc],
                        start=False, stop=True,
                    )
                red_jobs.append((si, tiles, padc, base, P))

            # DVE reduces in slot order; A-side chain interleaved in valid
            # dataflow order (emission order defines each engine's program
            # order AND must be topological wrt data deps).
            sqA = wpool.tile([128, W], F32, tag="sqA")
            ssA = wpool.tile([128, rt], F32, tag="ssA")
            nA = wpool.tile([128, rt], F32, tag="nA")
            rA = wpool.tile([128, rt], F32, tag="rA")
            abr = wpool.tile([128, rt], F32, tag="abr")
            rr = wpool.tile([128, rt], F32, tag="rr")

            def emit_red(job):
                si, tiles, padc, base, P = job
                nc.vector.tensor_reduce(
                    mval[:, base : base + tiles],
                    P[:, :].rearrange("p (t c) -> p t c", c=512)[:, 0:tiles, 0:padc],
                    axis=AX.X, op=ALU.max,
                )

            emit_red(red_jobs[0])
            emit_red(red_jobs[1])
            nc.scalar.activation(sqA[:, :half], tA[:, :half], AF.Square)
            nc.scalar.activation(sqA[:, half:], tA[:, half:], AF.Square)
            emit_red(red_jobs[2])
            nc.vector.tensor_reduce(
                ssA[:], sqA[:, :].rearrange("p (t d) -> p t d", d=D), axis=AX.X, op=ALU.add
            )
            nc.scalar.activation(nA[:], ssA[:], AF.Sqrt)
            nc.vector.reciprocal(rA[:], nA[:])
            nc.vector.tensor_reduce(
                abr[:], ab[:, :].rearrange("p (t d) -> p t d", d=D), axis=AX.X, op=ALU.add
            )
            nc.vector.tensor_tensor(rr[:], rA[:], rB[:], op=ALU.mult)
            # dist_pos branch (off the tail)
            cp = wpool.tile([128, rt], F32, tag="cp")
            nc.vector.tensor_tensor(cp[:], abr[:], rr[:], op=ALU.mult)
            cp3 = wpool.tile([128, rt], F32, tag="cp3")
            nc.vector.tensor_scalar(cp3[:], cp[:], 1.0, None, op0=ALU.min)
            dp = wpool.tile([128, rt], F32, tag="dp")
            nc.scalar.activation(dp[:], cp3[:], AF.Sqrt, bias=bias2[:, :], scale=-2.0)
            emit_red(red_jobs[3])
            # tail
            mc = wpool.tile([128, rt], F32, tag="mc")
            nc.vector.tensor_tensor(mc[:], mval[:], rA[:], op=ALU.mult)
            mc2 = wpool.tile([128, rt], F32, tag="mc2")
            nc.vector.tensor_scalar(mc2[:], mc[:], 1.0, None, op0=ALU.min)
            dn = wpool.tile([128, rt], F32, tag="dn")
            nc.scalar.activation(dn[:], mc2[:], AF.Sqrt, bias=bias2[:, :], scale=-2.0)
            if use_valid:
                valid = wpool.tile([128, rt], F32, tag="valid")
                nc.vector.tensor_scalar(valid[:], mval[:], -TH, None, op0=ALU.is_gt)
                dnv = wpool.tile([128, rt], F32, tag="dnv")
                nc.vector.tensor_tensor(dnv[:], dn[:], valid[:], op=ALU.mult)
                dn = dnv
            pre = wpool.tile([128, rt], F32, tag="pre")
            nc.vector.tensor_tensor(pre[:], dp[:], dn[:], op=ALU.subtract)
            losses = wpool.tile([128, rt], F32, tag="losses")
            nc.scalar.activation(losses[:], pre[:], AF.Relu, bias=MARGIN)
            nc.sync.dma_start(out=out[:, :], in_=losses[:])
            if dbg is not None:
                nc.sync.dma_start(out=dbg["mval"][:, :], in_=mval[:])
                nc.sync.dma_start(out=dbg["rA"][:, :], in_=rA[:])
                nc.sync.dma_start(out=dbg["rB"][:, :], in_=rB[:])
                nc.sync.dma_start(out=dbg["abr"][:, :], in_=abr[:])
                nc.sync.dma_start(out=dbg["dp"][:, :], in_=dp[:])
                nc.sync.dma_start(out=dbg["dn"][:, :], in_=dn[:])
                fbf = wpool.tile([33, RMAX], F32, tag="fbf")
                nc.scalar.copy(fbf[:], fb[:, :])
                nc.sync.dma_start(out=dbg["fb"][:, :], in_=fbf[:])

    _split_drain_waits(nc)
    return nc


def _kernel_v4(A, B, lab):
    """Column-tight packing: groups ranked by size; slot i of core c hosts the
    group with rank 8*i+c. Slot geometry (tiles, padc) shared by all cores."""
    counts = np.bincount(lab, minlength=G)
    gn = len(counts)
    if gn != 32 or counts.min() < 1:
        return None
    order_by_size = np.argsort(-counts, kind="stable")  # ranks -> group id
    nslots = gn // NCORES
    slots = []
    for si in range(nslots):
        ranks = order_by_size[si * NCORES : (si + 1) * NCORES]
        cmax = int(counts[ranks].max())
        tiles = -(-cmax // 128)
        padc = min(-(-cmax // 16) * 16, tiles * 128)
        if tiles > 4:
            return None
        slots.append((tiles, padc))
    rt = sum(t for t, _ in slots)
    if rt > 16:
        return None
    bases = np.cumsum([0] + [t for t, _ in slots])[:-1]
    RMAX = rt * 128

    order = np.argsort(lab, kind="stable")
    starts = np.concatenate([[0], np.cumsum(counts)])

    in_maps = []
    core_counts = []
    import ml_dtypes

    for c in range(NCORES):
        a_rows = np.ones((RMAX, D), np.float32)
        b_rows = np.ones((RMAX, D), np.float32)
        cv = np.zeros((1, RMAX), np.float32)
        ccounts = []
        for si in range(nslots):
            g = int(order_by_size[si * NCORES + c])
            n = int(counts[g])
            r0 = int(bases[si]) * 128
            rows = order[starts[g] : starts[g] + n]
            a_rows[r0 : r0 + n] = A[rows]
            b_rows[r0 : r0 + n] = B[rows]
            cv[0, r0 + n : r0 + slots[si][0] * 128] = -BIG
            ccounts.append(n)
        core_counts.append(ccounts)
        fa = np.concatenate(
            [a_rows.T, np.ones((1, RMAX), np.float32)], axis=0
        ).astype(ml_dtypes.bfloat16)
        in_maps.append(
            {
                "a": _tile_tp(a_rows),
                "b": _tile_tp(b_rows),
                "fa": np.ascontiguousarray(fa),
                "cv": np.ascontiguousarray(cv.astype(ml_dtypes.bfloat16)),
            }
        )

    use_valid = bool(counts.min() < 2)
    global _last_in_maps, _last_nc
    _last_in_maps = in_maps
    nc = _get_nc_v4(tuple(slots), use_valid)
    _last_nc = nc
    res = run_bass_kernel_spmd(nc, in_maps, list(range(NCORES)))
    total = 0.0
    for c in range(NCORES):
        lo = res.results[c]["losses"]  # [128, rt]; [p, T] = loss of row T*128+p
        flat = lo.T.reshape(RMAX)
        for si in range(nslots):
            n = core_counts[c][si]
            r0 = int(bases[si]) * 128
            total += float(flat[r0 : r0 + n].sum(dtype=np.float64))
    return np.float32(total / N)


_NC_CACHE = None
_NC_SORTED_CACHE = {}
_NC_V4_CACHE = {}


def _get_nc_v4(slots, use_valid):
    key = (slots, use_valid)
    if key not in _NC_V4_CACHE:
        _NC_V4_CACHE[key] = _build_nc_v4(slots, use_valid)
    return _NC_V4_CACHE[key]


def _get_nc():
    global _NC_CACHE
    if _NC_CACHE is None:
        _NC_CACHE = _build_nc()
    return _NC_CACHE


def _get_nc_sorted(gpc, padg):
    key = (gpc, padg)
    if key not in _NC_SORTED_CACHE:
        _NC_SORTED_CACHE[key] = _build_nc_sorted(gpc, padg)
    return _NC_SORTED_CACHE[key]


def _tile_tp(x):
    """[R, 32] rows -> [128, (R/128)*32] with row t*128+p on partition p."""
    r = x.shape[0]
    return (
        np.ascontiguousarray(
            x.reshape(r // 128, 128, D).transpose(1, 0, 2).reshape(128, (r // 128) * D)
        )
    )


def _kernel_sorted(A, B, lab):
    counts = np.bincount(lab, minlength=G)
    gn = len(counts)
    gpc = -(-gn // NCORES)
    padg = max(128, -(-int(counts.max()) // 128) * 128)
    if padg > 512:
        return None  # degenerate label distribution: fall back to full kernel
    rmax = gpc * padg
    rt = rmax // 128

    order = np.argsort(lab, kind="stable")
    starts = np.concatenate([[0], np.cumsum(counts)])

    src = np.full((NCORES, rmax), -1, np.int64)
    for g in range(gn):
        c, gl = divmod(g, gpc)
        n = int(counts[g])
        src[c, gl * padg : gl * padg + n] = order[starts[g] : starts[g] + n]

    in_maps = []
    for c in range(NCORES):
        idx = src[c]
        real = idx >= 0
        a_rows = np.ones((rmax, D), np.float32)
        b_rows = np.ones((rmax, D), np.float32)
        a_rows[real] = A[idx[real]]
        b_rows[real] = B[idx[real]]
        cv = np.ones((2, rmax), np.float32)
        cv[1] = np.where(real, 2.0, 2.0 + BIG)
        in_maps.append(
            {
                "a": _tile_tp(a_rows),
                "b": _tile_tp(b_rows),
                "cv": np.ascontiguousarray(cv),
            }
        )

    global _last_in_maps, _last_nc
    _last_in_maps = in_maps
    nc = _get_nc_sorted(gpc, padg)
    _last_nc = nc
    res = run_bass_kernel_spmd(nc, in_maps, list(range(NCORES)))
    total = 0.0
    for c in range(NCORES):
        lo = res.results[c]["losses"]  # [128, rt]; [p, t] = loss of local row t*128+p
        flat = lo.T.reshape(rmax)
        real = src[c] >= 0
        total += float(flat[real].sum(dtype=np.float64))
    return np.float32(total / N)


def kernel(A=None, B=None, labels=None, **_unused):
    import os

    A = np.asarray(A, dtype=np.float32)
    B = np.asarray(B, dtype=np.float32)
    lab = np.asarray(labels).astype(np.int64)
    lab = lab - lab.min() if lab.min() < 0 else lab

    if not os.environ.get("KERNEL_FORCE_FULL"):
        if not os.environ.get("KERNEL_FORCE_SORTED"):
            try:
                out = _kernel_v4(A, B, lab.astype(np.int32))
            except Exception:
                if os.environ.get("KERNEL_V4_STRICT"):
                    raise
                out = None
            if out is not None:
                return out
        out = _kernel_sorted(A, B, lab.astype(np.int32))
        if out is not None:
            return out

    eye = np.arange(G, dtype=np.int32)
    in_maps = []
    for c in range(NCORES):
        rows = slice(c * RPC, (c + 1) * RPC)
        a_c = _tile_tp(A[rows])
        b_rot = np.roll(B, -c * RPC, axis=0)
        lab_rot = np.roll(lab, -c * RPC)
        b_c = _tile_tp(b_rot)
        oha = np.concatenate(
            [
                np.ones((1, RPC), np.float32),
                (-BIG) * (lab[rows][None, :] == eye[:, None]).astype(np.float32),
            ]
        )
        ohb = np.concatenate(
            [
                np.full((1, N), 2.0 + BIG, np.float32),
                (lab_rot[None, :] == eye[:, None]).astype(np.float32),
            ]
        )
        in_maps.append(
            {
                "a": a_c,
                "b": b_c,
                "oha": np.ascontiguousarray(oha),
                "ohb": np.ascontiguousarray(ohb),
            }
        )

    global _last_in_maps, _last_nc
    _last_in_maps = in_maps
    nc = _get_nc()
    _last_nc = nc
    res = run_bass_kernel_spmd(nc, in_maps, list(range(NCORES)))
    total = 0.0
    for c in range(NCORES):
        lo = res.results[c]["losses"]  # [128, RT]; [p, r] = loss of row r*128+p
        total += float(lo.sum(dtype=np.float64))
    return np.float32(total / N)

